# revision 19
# baseline (speedup 1.0000x reference)
"""Trainium2 Bass kernel for a binarized 3-layer MLP (sign-binarize matmuls +
BatchNorm + hardtanh, final 2-class linear + log_softmax).

Strategy: pure data parallel over 8 NeuronCores (batch sharded), weights
replicated.  All matmuls contract over the feature dim, so activations are
kept feature-on-partition / batch-on-free on chip; the input x is staged
pre-transposed from the host (layout choice only - no host compute).

Since sign() values are +-1 exactly, the binary matmuls run on the PE in
fp8e4 with fp32 PSUM accumulation (bit-exact integer dots), all in DoubleRow
perf mode (256-deep contraction per pass; K padded with zero weights /
zero activations to a multiple of 256).  Weights are encoded +-0.5 (one
(is_gt, sub) vector op), activations +-1 (one ScalarE Sign op with the BN
affine folded in: sign(alpha*dot + beta)), so the dot comes out scaled by
1/2 and the 2x is folded into alpha.  Layer 3 keeps real values:
clip(alpha*psum + beta, -1, 1) in fp16.  The 2-class log_softmax collapses
to out = [-softplus(d), -softplus(-d)], d = (w4[1]-w4[0]) @ h3 + (b4[1]-b4[0]),
computed as one fp16 matmul with split-precision columns
[dw_hi, -dw_hi, 4096*dw_lo, -4096*dw_lo]; the softplus tail for all batch
tiles is batched at the end on a [2*n_nt, 512] tile.
"""

import sys
import types
from contextlib import ExitStack

import numpy as np

import concourse.bacc as bacc
import concourse.bass as bass
import concourse.tile as tile
from concourse import mybir
from concourse.bass_utils import run_bass_kernel_spmd

N_CORES = 8
B = 65536
B_PC = B // N_CORES  # 8192 rows per core
IN_F = 144
H = 1152
MT = H // 128  # 9 m-tiles of 128 output features
KT = 10        # k-slots (1152 padded to 1280 = 5 DoubleRow groups)
NT = 512       # batch tile (free dim)
BN_EPS = 1e-5

F32 = mybir.dt.float32
F16 = mybir.dt.float16
FP8 = mybir.dt.float8e4
DR = mybir.MatmulPerfMode.DoubleRow
AF = mybir.ActivationFunctionType
OP = mybir.AluOpType


def _install_ntff_hook():
    """Register the axon NTFF profiling hook if the image lacks
    antenv.axon_hooks (used only when tracing; harmless otherwise)."""
    try:
        import antenv
        if "antenv.axon_hooks" in sys.modules:
            return
        mod = types.ModuleType("antenv.axon_hooks")
        _h = [None]
        mod.set_axon_ntff_profile_hook = lambda h: _h.__setitem__(0, h)
        mod.get_axon_ntff_profile_hook = lambda: _h[0]
        sys.modules["antenv.axon_hooks"] = mod
        antenv.axon_hooks = mod
        from trn_agent_boot.trn_boot import _ntff_profile_via_ctypes
        mod.set_axon_ntff_profile_hook(
            _ntff_profile_via_ctypes("/opt/axon/libaxon_pjrt.so"))
    except Exception:
        pass


def build(b_pc=B_PC):
    nc = bacc.Bacc("TRN2", target_bir_lowering=False, debug=False,
                   num_devices=N_CORES)
    n_nt = b_pc // NT

    # ---- DRAM inputs (per core; weights replicated) ----
    xt = nc.dram_tensor("xt", [IN_F, b_pc], F32, kind="ExternalInput")
    w1t = nc.dram_tensor("w1t", [128, 2, H], F32, kind="ExternalInput")
    w2t = nc.dram_tensor("w2t", [128, KT, H], F32, kind="ExternalInput")
    w3t = nc.dram_tensor("w3t", [128, KT, H], F32, kind="ExternalInput")
    w4t = nc.dram_tensor("w4t", [128, MT, 2], F32, kind="ExternalInput")
    b4d = nc.dram_tensor("b4", [2], F32, kind="ExternalInput")
    cons = {}
    for l in (1, 2, 3):
        for nm in ("g", "be", "m", "v", "b"):
            cons[(nm, l)] = nc.dram_tensor(
                f"{nm}{l}r", [128, MT], F32, kind="ExternalInput")
    out = nc.dram_tensor("out", [b_pc, 2], F32, kind="ExternalOutput")

    with tile.TileContext(nc) as tc:
        with ExitStack() as ctx:
            wpool = ctx.enter_context(tc.tile_pool(name="wres", bufs=1))
            wstg = ctx.enter_context(tc.tile_pool(name="wstg", bufs=2))
            cpool = ctx.enter_context(tc.tile_pool(name="cons", bufs=1))
            xpool = ctx.enter_context(tc.tile_pool(name="xin", bufs=3))
            zpool = ctx.enter_context(tc.tile_pool(name="zs", bufs=3))
            hpool = ctx.enter_context(tc.tile_pool(name="h3", bufs=2))
            psum = ctx.enter_context(
                tc.tile_pool(name="psum", bufs=4, space="PSUM"))
            psumd = ctx.enter_context(
                tc.tile_pool(name="psumd", bufs=2, space="PSUM"))

            # persistent activation tensors (ping-pong) with zero-padded
            # k-slots that are memset once and never rewritten
            a1p = [wpool.tile([128, 2, NT], FP8, tag=f"a1_{i}", name=f"a1_{i}")
                   for i in range(2)]
            h1p = [wpool.tile([128, KT, NT], FP8, tag=f"h1_{i}", name=f"h1_{i}")
                   for i in range(2)]
            h2p = [wpool.tile([128, KT, NT], FP8, tag=f"h2_{i}", name=f"h2_{i}")
                   for i in range(2)]
            for t in a1p:
                nc.vector.memset(t[:, 1, :], 0.0)
            for t in h1p + h2p:
                nc.vector.memset(t[:, 9, :], 0.0)
            dall = wpool.tile([2 * n_nt, NT], F32, tag="dall")
            dlo = wpool.tile([2 * n_nt, NT], F32, tag="dlo")
            bias32 = wpool.tile([2 * n_nt, 1], F32, tag="bias32")

            # ---- one-time weight prep: w -> +-0.5 in fp8 ----
            # (is_gt 0, sub 0.5); zero-padded regions in w1t turn into -0.5
            # but multiply only all-zero activation pad slots, so stay inert.
            w1s = wpool.tile([128, 2, H], FP8, tag="w1s")
            w2s = wpool.tile([128, KT, H], FP8, tag="w2s")
            w3s = wpool.tile([128, KT, H], FP8, tag="w3s")
            nc.vector.memset(w2s[:, 9, :], 0.0)
            nc.vector.memset(w3s[:, 9, :], 0.0)
            stg = wstg.tile([128, 2 * H], F32, tag="wstg")
            nc.gpsimd.dma_start(stg[:], w1t.ap().rearrange("p i h -> p (i h)"))
            nc.vector.tensor_scalar(w1s.rearrange("p i h -> p (i h)"), stg[:],
                                    0.0, 0.5, OP.is_gt, OP.subtract)
            for j in range(MT):
                stg = wstg.tile([128, H], F32, tag="wstg2")
                nc.gpsimd.dma_start(stg[:], w2t.ap()[:, j, :])
                nc.vector.tensor_scalar(w2s[:, j, :], stg[:],
                                        0.0, 0.5, OP.is_gt, OP.subtract)
            for j in range(MT):
                stg = wstg.tile([128, H], F32, tag="wstg3")
                nc.gpsimd.dma_start(stg[:], w3t.ap()[:, j, :])
                nc.gpsimd.tensor_scalar(w3s[:, j, :], stg[:],
                                        0.0, 0.5, OP.is_gt, OP.subtract)
            # slot 9 of w2s/w3s multiplies the zeroed slot 9 of h1/h2; w1s
            # slot-1 rows 16.. multiply the zeroed rows of a1.

            # ---- final-layer split-precision diff weights (fp16) ----
            # cols: [dw_hi, -dw_hi, 4096*dw_lo, -4096*dw_lo]
            w4st = cpool.tile([128, MT, 2], F32)
            nc.gpsimd.dma_start(w4st.rearrange("p k c -> p (k c)"),
                                w4t.ap().rearrange("p k c -> p (k c)"))
            dwf = cpool.tile([128, MT], F32)
            nc.vector.tensor_tensor(dwf[:], w4st[:, :, 1], w4st[:, :, 0],
                                    OP.subtract)
            dwx = cpool.tile([128, MT, 4], F16)
            nc.vector.tensor_copy(dwx[:, :, 0], dwf[:])
            nc.vector.tensor_scalar_mul(dwx[:, :, 1], dwx[:, :, 0], -1.0)
            lof = cpool.tile([128, MT], F32)
            nc.vector.tensor_tensor(lof[:], dwf[:], dwx[:, :, 0], OP.subtract)
            nc.vector.tensor_scalar_mul(dwx[:, :, 2], lof[:], 4096.0)
            nc.vector.tensor_scalar_mul(dwx[:, :, 3], dwx[:, :, 2], -1.0)

            # softplus bias rows [b4[1]-b4[0], b4[0]-b4[1]], replicated
            tb = cpool.tile([2, 1], F32)
            tbr = cpool.tile([2, 1], F32)
            nc.gpsimd.dma_start(tb[:], b4d.ap().rearrange("(c o) -> c o", o=1))
            nc.gpsimd.dma_start(tbr[0:1, :],
                                b4d.ap()[1:2].rearrange("(c o) -> c o", o=1))
            nc.gpsimd.dma_start(tbr[1:2, :],
                                b4d.ap()[0:1].rearrange("(c o) -> c o", o=1))
            bias2 = cpool.tile([2, 1], F32)
            nc.vector.tensor_tensor(bias2[:], tbr[:], tb[:], OP.subtract)
            for j in range(n_nt):
                nc.sync.dma_start(bias32[2 * j:2 * j + 2, :], bias2[:])

            # ---- BN folds: alpha = g*rsqrt(v+eps); beta = alpha*(b-m)+be;
            # scale used on-chip is 2*alpha (weights are +-0.5) ----
            alpha2 = {}
            beta = {}
            for l in (1, 2, 3):
                ct = {}
                for nm in ("g", "be", "m", "v", "b"):
                    t = cpool.tile([128, MT], F32, tag=f"c{nm}{l}")
                    nc.gpsimd.dma_start(t[:], cons[(nm, l)].ap()[:, :])
                    ct[nm] = t
                a = cpool.tile([128, MT], F32, tag=f"alpha{l}")
                a2 = cpool.tile([128, MT], F32, tag=f"alpha2_{l}")
                bt = cpool.tile([128, MT], F32, tag=f"beta{l}")
                tmp = cpool.tile([128, MT], F32, tag=f"tmp{l}")
                nc.vector.tensor_scalar_add(tmp[:], ct["v"][:], BN_EPS)
                nc.scalar.activation(tmp[:], tmp[:], AF.Sqrt)
                nc.vector.reciprocal(a[:], tmp[:])
                nc.vector.tensor_tensor(a[:], a[:], ct["g"][:], OP.mult)
                nc.vector.tensor_tensor(tmp[:], ct["b"][:], ct["m"][:],
                                        OP.subtract)
                nc.vector.tensor_tensor(tmp[:], tmp[:], a[:], OP.mult)
                nc.vector.tensor_tensor(bt[:], tmp[:], ct["be"][:], OP.add)
                nc.vector.tensor_scalar_mul(a2[:], a[:], 2.0)
                alpha2[l] = a2
                beta[l] = bt

            # ---- main batch loop ----
            for n in range(n_nt):
                ncols = bass.ts(n, NT)
                a1 = a1p[n % 2]
                h1 = h1p[n % 2]
                h2 = h2p[n % 2]

                # L1 input: sign(x), feature-on-partition, K padded to 256
                xa = xpool.tile([128, NT], F32, tag="xa")
                nc.sync.dma_start(xa[:], xt.ap()[0:128, ncols])
                xb = xpool.tile([16, NT], F32, tag="xb")
                nc.sync.dma_start(xb[:], xt.ap()[128:IN_F, ncols])
                nc.scalar.activation(a1[:, 0, :], xa[:], AF.Sign)
                nc.scalar.activation(a1[0:16, 1, :], xb[:], AF.Sign)

                # L1: one DoubleRow matmul per m-tile
                for m in range(MT):
                    ps = psum.tile([128, NT], F32)
                    nc.tensor.matmul(ps[:], w1s[:, :, bass.ts(m, 128)],
                                     a1[:, :, :], start=True, stop=True,
                                     perf_mode=DR)
                    nc.scalar.activation(h1[:, m, :], ps[:], AF.Sign,
                                         bias=beta[1][:, m:m + 1],
                                         scale=alpha2[1][:, m:m + 1])

                # L2 / L3: 5 DoubleRow groups each
                h3 = hpool.tile([128, MT, NT], F16, tag="h3")
                for l, ws, src in ((2, w2s, h1), (3, w3s, h2)):
                    for m in range(MT):
                        ps = psum.tile([128, NT], F32)
                        mcols = bass.ts(m, 128)
                        for g in range(5):
                            nc.tensor.matmul(
                                ps[:], ws[:, 2 * g:2 * g + 2, mcols],
                                src[:, 2 * g:2 * g + 2, :],
                                start=(g == 0), stop=(g == 4), perf_mode=DR)
                        if l == 2:
                            nc.scalar.activation(h2[:, m, :], ps[:], AF.Sign,
                                                 bias=beta[l][:, m:m + 1],
                                                 scale=alpha2[l][:, m:m + 1])
                        else:
                            zc = zpool.tile([128, NT], F32, tag="zc")
                            nc.vector.tensor_scalar(
                                zc[:], ps[:],
                                alpha2[l][:, m:m + 1], beta[l][:, m:m + 1],
                                OP.mult, OP.add)
                            nc.vector.tensor_scalar(
                                h3[:, m, :], zc[:], -1.0, 1.0,
                                OP.max, OP.min)

                # final: d rows [hi+, hi-, 4096lo+, 4096lo-] (fp16 matmul)
                dps = psumd.tile([4, NT], F32)
                for k in range(MT):
                    nc.tensor.matmul(dps[:], dwx[:, k, :], h3[:, k, :],
                                     start=(k == 0), stop=(k == MT - 1))
                q4 = zpool.tile([4, NT], F32, tag="q4")
                nc.vector.tensor_copy(q4[:], dps[:])
                nc.sync.dma_start(dall[2 * n:2 * n + 2, :], q4[0:2, :])
                nc.sync.dma_start(dlo[2 * n:2 * n + 2, :], q4[2:4, :])

            # ---- batched softplus tail: out = -(relu(z) + ln(1+e^-|z|)) ----
            P2 = 2 * n_nt
            tlo = wpool.tile([P2, NT], F32, tag="tlo")
            nc.vector.tensor_scalar(tlo[:], dlo[:], 1.0 / 4096.0,
                                    bias32[:, 0:1], OP.mult, OP.add)
            zb = wpool.tile([P2, NT], F32, tag="zb")
            nc.vector.tensor_tensor(zb[:], dall[:], tlo[:], OP.add)
            ab = wpool.tile([P2, NT], F32, tag="ab")
            nc.scalar.activation(ab[:], zb[:], AF.Abs)
            eb = wpool.tile([P2, NT], F32, tag="eb")
            nc.scalar.activation(eb[:], ab[:], AF.Exp, scale=-1.0)
            lb = wpool.tile([P2, NT], F32, tag="lb")
            nc.scalar.activation(lb[:], eb[:], AF.Ln, bias=1.0)
            rb = wpool.tile([P2, NT], F32, tag="rb")
            nc.vector.tensor_scalar(rb[:], zb[:], 0.0, None, OP.max)
            sb = wpool.tile([P2, NT], F32, tag="sb")
            nc.vector.tensor_tensor(sb[:], rb[:], lb[:], OP.add)
            ob = wpool.tile([P2, NT], F32, tag="ob")
            nc.vector.tensor_scalar_mul(ob[:], sb[:], -1.0)
            obv = ob.rearrange("(j c) n -> c j n", c=2)
            for c in range(2):
                nc.sync.dma_start(
                    out.ap()[:, c:c + 1].rearrange("(j n) o -> j (n o)", n=NT),
                    obv[c])

    nc.compile()
    return nc


_CACHE = {}


def _get_nc(b_pc):
    if b_pc not in _CACHE:
        _CACHE[b_pc] = build(b_pc)
    return _CACHE[b_pc]


def _prep_shared(w1, w2, w3, w4, b4, bn):
    """Host-side pure relayouts of weights/constants (no arithmetic on
    values; padding with zeros only)."""
    d = {}
    w1p = np.zeros((256, H), dtype=np.float32)
    w1p[:IN_F] = np.ascontiguousarray(w1.T)
    d["w1t"] = np.ascontiguousarray(w1p.reshape(2, 128, H).transpose(1, 0, 2))
    for nm, w in (("w2t", w2), ("w3t", w3)):
        wp = np.zeros((KT * 128, H), dtype=np.float32)
        wp[:H] = np.ascontiguousarray(w.T)
        d[nm] = np.ascontiguousarray(
            wp.reshape(KT, 128, H).transpose(1, 0, 2))
    d["w4t"] = np.ascontiguousarray(
        np.ascontiguousarray(w4.T).reshape(MT, 128, 2).transpose(1, 0, 2))
    d["b4"] = np.ascontiguousarray(b4)
    for l in (1, 2, 3):
        for nm in ("g", "be", "m", "v", "b"):
            d[f"{nm}{l}r"] = np.ascontiguousarray(
                bn[(nm, l)].reshape(MT, 128).T)
    return d


def _run(inputs, trace=False, b_pc=B_PC, tmpdir=None):
    x = inputs["x"]
    bn = {}
    for l in (1, 2, 3):
        for nm, key in (("g", f"g{l}"), ("be", f"be{l}"), ("m", f"m{l}"),
                        ("v", f"v{l}"), ("b", f"b{l}")):
            bn[(nm, l)] = np.asarray(inputs[key], dtype=np.float32)
    shared = _prep_shared(
        np.asarray(inputs["w1"], np.float32), np.asarray(inputs["w2"], np.float32),
        np.asarray(inputs["w3"], np.float32), np.asarray(inputs["w4"], np.float32),
        np.asarray(inputs["b4"], np.float32), bn)

    xT = np.ascontiguousarray(np.asarray(x, np.float32).T)  # [144, B]
    n_use = xT.shape[1] // b_pc
    assert n_use == N_CORES, (xT.shape, b_pc)
    in_maps = []
    for c in range(N_CORES):
        m = dict(shared)
        m["xt"] = np.ascontiguousarray(xT[:, c * b_pc:(c + 1) * b_pc])
        in_maps.append(m)

    nc = _get_nc(b_pc)
    if trace:
        _install_ntff_hook()
    res = run_bass_kernel_spmd(nc, in_maps, list(range(N_CORES)), trace=trace,
                               tmpdir=tmpdir)
    outs = [res.results[c]["out"] for c in range(N_CORES)]
    full = np.concatenate(outs, axis=0)
    return full, res.exec_time_ns


def kernel(**inputs):
    out, _ = _run(inputs, trace=False)
    return out
